# revision 4
# baseline (speedup 1.0000x reference)
"""Trainium2 Bass kernel for CustomMHA (bsz=4, seq=2048, embd=1024, nhead=16).

The reference's "buggy" head split
    q = query.reshape(T, B*H, hd).swapaxes(0, 1)
is equivalent to de-interleaving rows of the (B*T, E) 2-D view mod 4 into 4
row-groups rb, then a standard 16-head split of the 1024 columns within each
group.  The 64 (rb, cb) heads are sharded 8 per core: core c = rb*2 + half
owns row-group rb and columns cols = [half*512, half*512+512).

Per core (matmuls in float32r = fp32 RNE-rounded to 11 mantissa bits, which
streams at 1 cycle/row on the PE; inputs are pre-rounded on the host so they
can be DMAed straight into f32r tiles):
  K_T[c,s]  = sum_e kwT[e,c] * keyT[e,s] + kb[c]     (c on partitions)
  V[s,c]    = sum_e valueT[e,s] * vw[c,e] + vb[c]    (s on partitions)
  per head: S_T[s,t] = sum_d K_T[hd,s] * qT[hd,t]    (no sqrt(hd) scaling)
            P~ = exp(S_T); r[t] = 1/sum_s P~[s,t]    (gpsimd partition_all_reduce)
            P  = P~ * r   -> DRAM (s,t layout; host transposes)
            O_T[d,t] = sum_s V[s,d] * P[s,t]
  outp[t,j] = sum_c O_T[c,t] * w2T[c,j]              (partial; host sums pairs)
"""
import sys

sys.path.insert(0, "/opt/trn_rl_repo")

import numpy as np

BSZ, SEQ, EMBD, NHEAD = 4, 2048, 1024, 16
HD = EMBD // NHEAD          # 64
CLOC = 512                  # local columns per core (8 heads)
NCT = CLOC // 128           # 4
NET = EMBD // 128           # 8
NST = SEQ // 128            # 16
TBLK = 512
NTB = SEQ // TBLK           # 4
NHL = 8                     # local heads per core

_BASS = None


def _round_f32r(x):
    """RNE to 11 mantissa bits == hardware f32r rounding (verified on HW)."""
    xi = np.ascontiguousarray(x, dtype=np.float32).view(np.uint32).astype(np.uint64)
    shift = 12
    bias = ((xi >> shift) & 1) + (1 << (shift - 1)) - 1
    xi = (xi + bias) >> shift << shift
    return xi.astype(np.uint32).view(np.float32)


def _build_bass():
    import concourse.mybir as mybir
    import concourse.tile as tile
    import concourse.bacc as bacc
    from concourse import bass_isa

    f32 = mybir.dt.float32
    f32r = mybir.dt.float32r
    AF = mybir.ActivationFunctionType
    Alu = mybir.AluOpType

    nc = bacc.Bacc("TRN2", target_bir_lowering=False, debug=False)

    qT_d = nc.declare_dram_parameter("qT", [CLOC, SEQ], f32r, isOutput=False)
    keyT_d = nc.declare_dram_parameter("keyT", [EMBD, SEQ], f32r, isOutput=False)
    valueT_d = nc.declare_dram_parameter("valueT", [EMBD, SEQ], f32r, isOutput=False)
    kwT_d = nc.declare_dram_parameter("kwT", [EMBD, CLOC], f32r, isOutput=False)
    vwT_d = nc.declare_dram_parameter("vwT", [EMBD, CLOC], f32r, isOutput=False)
    w2T_d = nc.declare_dram_parameter("w2T", [CLOC, EMBD], f32r, isOutput=False)
    kb_d = nc.declare_dram_parameter("kb", [128, NCT], f32, isOutput=False)
    vb_d = nc.declare_dram_parameter("vb", [1, CLOC], f32, isOutput=False)
    attn_d = nc.declare_dram_parameter("attn_t", [NHL, SEQ, SEQ], f32, isOutput=True)
    outp_d = nc.declare_dram_parameter("outp", [SEQ, EMBD], f32, isOutput=True)

    with tile.TileContext(nc) as tc:
        with (
            tc.tile_pool(name="persist", bufs=1) as persist,
            tc.tile_pool(name="wpool", bufs=1) as wpool,
            tc.tile_pool(name="stream", bufs=9) as stream,
            tc.tile_pool(name="qpool", bufs=2) as qpool,
            tc.tile_pool(name="work", bufs=2) as work,
            tc.tile_pool(name="pnpool", bufs=3) as pnpool,
            tc.tile_pool(name="pexpp", bufs=1) as pexpp,
            tc.tile_pool(name="psum_a", bufs=5, space="PSUM") as psum_a,
            tc.tile_pool(name="psum_o", bufs=2, space="PSUM") as psum_o,
        ):
            # ---------------- persistent tiles ----------------
            KT = persist.tile([128, NCT, SEQ], f32r, tag="KT")           # 32 KB/p
            V = persist.tile([128, NST, CLOC], f32r, tag="V")            # 32 KB/p
            O_sb = persist.tile([128, NCT, SEQ], f32r, tag="O_sb")       # 32 KB/p
            kb_sb = persist.tile([128, NCT], f32, tag="kb")
            vb_sb = persist.tile([128, CLOC], f32, tag="vb")

            nc.sync.dma_start(kb_sb[:], kb_d[:])
            nc.sync.dma_start(vb_sb[:], vb_d[:].to_broadcast((128, CLOC)))

            # ---------------- K projection ----------------
            kwT_r = wpool.tile([128, NET, CLOC], f32r, tag="wr")
            nc.sync.dma_start(kwT_r[:], kwT_d[:].rearrange("(et p) c -> p et c", p=128))

            for sblk in range(4):
                ksl = []
                for et in range(NET):
                    kr = stream.tile([128, 512], f32r, tag="kvslice")
                    nc.sync.dma_start(
                        kr[:], keyT_d[et * 128:(et + 1) * 128, sblk * 512:(sblk + 1) * 512]
                    )
                    ksl.append(kr)
                for ct in range(NCT):
                    ps = psum_a.tile([128, 512], f32)
                    for et in range(NET):
                        nc.tensor.matmul(
                            ps[:],
                            kwT_r[:, et, ct * 128:(ct + 1) * 128],
                            ksl[et][:],
                            start=(et == 0), stop=(et == NET - 1),
                        )
                    # add key bias (per-partition) and round to f32r
                    nc.scalar.activation(
                        KT[:, ct, sblk * 512:(sblk + 1) * 512], ps[:],
                        AF.Identity, bias=kb_sb[:, ct:ct + 1], scale=1.0,
                    )

            # ---------------- V projection ----------------
            vwT_r = wpool.tile([128, NET, CLOC], f32r, tag="wr")
            nc.sync.dma_start(vwT_r[:], vwT_d[:].rearrange("(et p) c -> p et c", p=128))

            for stg in range(4):               # groups of 4 s-tiles
                vsl = []
                for et in range(NET):
                    vr = stream.tile([128, 512], f32r, tag="kvslice")
                    nc.sync.dma_start(
                        vr[:], valueT_d[et * 128:(et + 1) * 128, stg * 512:(stg + 1) * 512]
                    )
                    vsl.append(vr)
                for sj in range(4):
                    st = stg * 4 + sj
                    ps = psum_a.tile([128, CLOC], f32)
                    for et in range(NET):
                        nc.tensor.matmul(
                            ps[:],
                            vsl[et][:, sj * 128:(sj + 1) * 128],
                            vwT_r[:, et, :],
                            start=(et == 0), stop=(et == NET - 1),
                        )
                    # add value bias (broadcast over partitions) + round to f32r
                    nc.vector.tensor_tensor(V[:, st, :], ps[:], vb_sb[:], Alu.add)

            # ---------------- attention ----------------
            for ct in range(NCT):
                qct = qpool.tile([128, SEQ], f32r, tag="qct")
                nc.sync.dma_start(qct[:], qT_d[ct * 128:(ct + 1) * 128, :])
                for hp in range(2):
                    hl = ct * 2 + hp
                    prange = slice(hp * 64, hp * 64 + 64)
                    for tb in range(NTB):
                        tsl = slice(tb * TBLK, (tb + 1) * TBLK)
                        pexp = pexpp.tile([128, NST, TBLK], f32, tag="pexp")
                        acc = work.tile([128, TBLK], f32, tag="acc")
                        for st in range(NST):
                            ps = psum_a.tile([128, TBLK], f32)
                            nc.tensor.matmul(
                                ps[:],
                                KT[prange, ct, st * 128:(st + 1) * 128],
                                qct[prange, tsl],
                                start=True, stop=True,
                            )
                            nc.scalar.activation(pexp[:, st, :], ps[:], AF.Exp)
                            if st == 0:
                                nc.vector.tensor_copy(acc[:], pexp[:, st, :])
                            else:
                                nc.vector.tensor_add(acc[:], acc[:], pexp[:, st, :])
                        rall = work.tile([128, TBLK], f32, tag="rall")
                        nc.gpsimd.partition_all_reduce(
                            rall[:], acc[:], channels=128,
                            reduce_op=bass_isa.ReduceOp.add,
                        )
                        R = work.tile([128, TBLK], f32, tag="R")
                        nc.vector.reciprocal(R[:], rall[:])

                        pso = psum_o.tile([64, TBLK], f32)
                        for st in range(NST):
                            pn = pnpool.tile([128, TBLK], f32r, tag="pn")
                            nc.vector.tensor_tensor(
                                pn[:], pexp[:, st, :], R[:], Alu.mult
                            )
                            nc.sync.dma_start(
                                attn_d[hl, st * 128:(st + 1) * 128, tsl],
                                pn[:].bitcast(f32),
                            )
                            nc.tensor.matmul(
                                pso[:],
                                V[:, st, hl * HD:(hl + 1) * HD],
                                pn[:],
                                start=(st == 0), stop=(st == NST - 1),
                            )
                        nc.vector.tensor_copy(O_sb[prange, ct, tsl], pso[:])

            # ---------------- output projection ----------------
            w2T_r = wpool.tile([128, NCT, EMBD], f32r, tag="wr")
            nc.sync.dma_start(w2T_r[:], w2T_d[:].rearrange("(ct p) j -> p ct j", p=128))

            for tt in range(NST):
                osb = work.tile([128, EMBD], f32, tag="osb")
                for jb in range(2):
                    ps = psum_a.tile([128, 512], f32)
                    for ct in range(NCT):
                        nc.tensor.matmul(
                            ps[:],
                            O_sb[:, ct, tt * 128:(tt + 1) * 128],
                            w2T_r[:, ct, jb * 512:(jb + 1) * 512],
                            start=(ct == 0), stop=(ct == NCT - 1),
                        )
                    nc.scalar.copy(osb[:, jb * 512:(jb + 1) * 512], ps[:])
                nc.sync.dma_start(outp_d[tt * 128:(tt + 1) * 128, :], osb[:])

    nc.compile()
    return nc


def _get_bass():
    global _BASS
    if _BASS is None:
        _BASS = _build_bass()
    return _BASS


_last_in_maps = None


def kernel(query, key, value, key_w, key_b, value_w, value_b, out_w, out_b,
           nhead=16, **_unused):
    from concourse.bass_utils import run_bass_kernel_spmd

    q = np.ascontiguousarray(np.asarray(query, dtype=np.float32))
    k = np.ascontiguousarray(np.asarray(key, dtype=np.float32))
    v = np.ascontiguousarray(np.asarray(value, dtype=np.float32))
    kw = np.asarray(key_w, dtype=np.float32)
    kb = np.asarray(key_b, dtype=np.float32)
    vw = np.asarray(value_w, dtype=np.float32)
    vb = np.asarray(value_b, dtype=np.float32)
    ow = np.asarray(out_w, dtype=np.float32)
    ob = np.asarray(out_b, dtype=np.float32)

    Q3 = q.reshape(SEQ, 4, EMBD)
    K3 = k.reshape(SEQ, 4, EMBD)
    V3 = v.reshape(SEQ, 4, EMBD)

    in_maps = []
    for core in range(8):
        rb, half = core // 2, core % 2
        cols = slice(half * CLOC, half * CLOC + CLOC)
        in_maps.append({
            "qT": _round_f32r(Q3[:, rb, cols].T),
            "keyT": _round_f32r(K3[:, rb, :].T),
            "valueT": _round_f32r(V3[:, rb, :].T),
            "kwT": _round_f32r(kw[cols, :].T),
            "vwT": _round_f32r(vw[cols, :].T),
            "w2T": _round_f32r(ow[:, cols].T),
            "kb": np.ascontiguousarray(kb[cols].reshape(NCT, 128).T),
            "vb": vb[cols].reshape(1, CLOC).copy(),
        })

    global _last_in_maps
    _last_in_maps = in_maps
    nc = _get_bass()
    res = run_bass_kernel_spmd(nc, in_maps, list(range(8)))

    out2d = np.empty((BSZ * SEQ, EMBD), np.float32)
    attn = np.empty((BSZ * NHEAD, SEQ, SEQ), np.float32)
    for rb in range(4):
        p0 = res.results[2 * rb]["outp"]
        p1 = res.results[2 * rb + 1]["outp"]
        out2d[rb::4] = p0 + p1 + ob[None, :]
        for half in range(2):
            at = res.results[2 * rb + half]["attn_t"]
            for hl in range(NHL):
                bh = rb * 16 + half * 8 + hl
                attn[bh] = at[hl].T
    out = out2d.reshape(BSZ, SEQ, EMBD)
    return out, attn


# revision 6
# speedup vs baseline: 1.6587x; 1.6587x over previous
"""Trainium2 Bass kernel for CustomMHA (bsz=4, seq=2048, embd=1024, nhead=16).

The reference's "buggy" head split
    q = query.reshape(T, B*H, hd).swapaxes(0, 1)
is equivalent to de-interleaving rows of the (B*T, E) 2-D view mod 4 into 4
row-groups rb, then a standard 16-head split of the 1024 columns within each
group.  The 64 (rb, cb) heads are sharded 8 per core: core c = rb*2 + half
owns row-group rb and columns cols = [half*512, half*512+512).

Per core (matmuls in float32r = fp32 RNE-rounded to 11 mantissa bits, 1
cycle/row on the PE; inputs pre-rounded on the host and DMAed straight into
f32r tiles):
  K_T[c,s]  = sum_e kwT[e,c] * keyT[e,s] + kb[c]     (c on partitions)
  V[s,c]    = sum_e valueT[e,s] * vw[c,e] + vb[c]    (s on partitions,
              65-wide per-head blocks whose last column is 1.0)
  per head pair (A=2ct, B=2ct+1), streaming over s-tiles:
    S_T[s,t] = sum_d K_T[hd,s] * qT[hd,t]   (row-packed pair, no hd scaling)
    P~ = exp(S_T)  -> DRAM unnormalized     (s,t layout; host transposes)
    pso[0:64]  = sum_s V[s,d] * P~[s,t]     (unnormalized O~)
    pso[64]    = sum_s P~[s,t]              (denominator, via the ones col)
    r = 1/pso[64] -> DRAM (host multiplies attn); O = O~ * r on device
  outp[t,j] = sum_c O[c,t] * w2T[c,j]       (partial; host sums core pairs)
"""
import sys

sys.path.insert(0, "/opt/trn_rl_repo")

import numpy as np

BSZ, SEQ, EMBD, NHEAD = 4, 2048, 1024, 16
HD = EMBD // NHEAD          # 64
CLOC = 512                  # local columns per core (8 heads)
NCT = CLOC // 128           # 4
NET = EMBD // 128           # 8
NST = SEQ // 128            # 16
TBLK = 512
NTB = SEQ // TBLK           # 4
NHL = 8                     # local heads per core
VW = HD + 1                 # 65: per-head V block width (ones column last)

_BASS = None


def _round_f32r(x):
    """RNE to 11 mantissa bits == hardware f32r rounding (verified on HW)."""
    xi = np.ascontiguousarray(x, dtype=np.float32).view(np.uint32).astype(np.uint64)
    shift = 12
    bias = ((xi >> shift) & 1) + (1 << (shift - 1)) - 1
    xi = (xi + bias) >> shift << shift
    return xi.astype(np.uint32).view(np.float32)


def _build_bass():
    import concourse.mybir as mybir
    import concourse.tile as tile
    import concourse.bacc as bacc

    f32 = mybir.dt.float32
    f32r = mybir.dt.float32r
    AF = mybir.ActivationFunctionType
    Alu = mybir.AluOpType

    nc = bacc.Bacc("TRN2", target_bir_lowering=False, debug=False)

    qT_d = nc.declare_dram_parameter("qT", [CLOC, SEQ], f32r, isOutput=False)
    keyT_d = nc.declare_dram_parameter("keyT", [EMBD, SEQ], f32r, isOutput=False)
    valueT_d = nc.declare_dram_parameter("valueT", [EMBD, SEQ], f32r, isOutput=False)
    kwT_d = nc.declare_dram_parameter("kwT", [EMBD, CLOC], f32r, isOutput=False)
    vwT_d = nc.declare_dram_parameter("vwT", [EMBD, CLOC], f32r, isOutput=False)
    w2T_d = nc.declare_dram_parameter("w2T", [CLOC, EMBD], f32r, isOutput=False)
    kb_d = nc.declare_dram_parameter("kb", [128, NCT], f32, isOutput=False)
    vb_d = nc.declare_dram_parameter("vb", [1, CLOC], f32, isOutput=False)
    attn_d = nc.declare_dram_parameter("attn_t", [NHL, SEQ, SEQ], f32, isOutput=True)
    r_d = nc.declare_dram_parameter("rrow", [NHL, SEQ], f32, isOutput=True)
    outp_d = nc.declare_dram_parameter("outp", [SEQ, EMBD], f32, isOutput=True)

    with tile.TileContext(nc) as tc:
        with (
            tc.tile_pool(name="persist", bufs=1) as persist,
            tc.tile_pool(name="wpool", bufs=1) as wpool,
            tc.tile_pool(name="stream", bufs=9) as stream,
            tc.tile_pool(name="qpool", bufs=2) as qpool,
            tc.tile_pool(name="work", bufs=2) as work,
            tc.tile_pool(name="pexpool", bufs=10) as pexpool,
            tc.tile_pool(name="psum_a", bufs=5, space="PSUM") as psum_a,
            tc.tile_pool(name="psum_o", bufs=3, space="PSUM") as psum_o,
        ):
            # ---------------- persistent tiles ----------------
            KT = persist.tile([128, NCT, SEQ], f32r, tag="KT")           # 32 KB/p
            V = persist.tile([128, NST, NHL * VW], f32r, tag="V")        # 32.5 KB/p
            O_sb = persist.tile([128, NCT, SEQ], f32r, tag="O_sb")       # 32 KB/p
            kb_sb = persist.tile([128, NCT], f32, tag="kb")
            vb_sb = persist.tile([128, CLOC], f32, tag="vb")

            nc.sync.dma_start(kb_sb[:], kb_d[:])
            nc.sync.dma_start(vb_sb[:], vb_d[:].to_broadcast((128, CLOC)))

            # ones column of every per-head V block (ACT writes 1.0, f32r)
            ones_view = V[:].rearrange("p s (h x) -> p s h x", x=VW)[:, :, :, HD:VW]
            nc.scalar.activation(
                ones_view,
                vb_sb[:, 0:1, None].to_broadcast(ones_view.shape),
                AF.Copy, bias=1.0, scale=0.0,
            )

            # ---------------- K projection ----------------
            kwT_r = wpool.tile([128, NET, CLOC], f32r, tag="wr")
            nc.sync.dma_start(kwT_r[:], kwT_d[:].rearrange("(et p) c -> p et c", p=128))

            for sblk in range(4):
                ksl = []
                for et in range(NET):
                    kr = stream.tile([128, 512], f32r, tag="kvslice")
                    nc.sync.dma_start(
                        kr[:], keyT_d[et * 128:(et + 1) * 128, sblk * 512:(sblk + 1) * 512]
                    )
                    ksl.append(kr)
                for ct in range(NCT):
                    ps = psum_a.tile([128, 512], f32, tag="ps")
                    for et in range(NET):
                        nc.tensor.matmul(
                            ps[:],
                            kwT_r[:, et, ct * 128:(ct + 1) * 128],
                            ksl[et][:],
                            start=(et == 0), stop=(et == NET - 1),
                        )
                    # add key bias (per-partition scalar) and round to f32r
                    nc.vector.tensor_scalar(
                        KT[:, ct, sblk * 512:(sblk + 1) * 512], ps[:],
                        kb_sb[:, ct:ct + 1], None, Alu.add,
                    )

            # ---------------- V projection ----------------
            vwT_r = wpool.tile([128, NET, CLOC], f32r, tag="wr")
            nc.sync.dma_start(vwT_r[:], vwT_d[:].rearrange("(et p) c -> p et c", p=128))

            for stg in range(4):               # groups of 4 s-tiles
                vsl = []
                for et in range(NET):
                    vr = stream.tile([128, 512], f32r, tag="kvslice")
                    nc.sync.dma_start(
                        vr[:], valueT_d[et * 128:(et + 1) * 128, stg * 512:(stg + 1) * 512]
                    )
                    vsl.append(vr)
                for sj in range(4):
                    st = stg * 4 + sj
                    ps = psum_a.tile([128, CLOC], f32, tag="ps")
                    for et in range(NET):
                        nc.tensor.matmul(
                            ps[:],
                            vsl[et][:, sj * 128:(sj + 1) * 128],
                            vwT_r[:, et, :],
                            start=(et == 0), stop=(et == NET - 1),
                        )
                    # bias add + scatter into 65-wide per-head blocks (f32r)
                    vdst = V[:, st, :].rearrange("p (h x) -> p h x", x=VW)[:, :, 0:HD]
                    nc.vector.tensor_tensor(
                        vdst,
                        ps[:].rearrange("p (h x) -> p h x", x=HD),
                        vb_sb[:].rearrange("p (h x) -> p h x", x=HD),
                        Alu.add,
                    )

            # ---------------- attention (head pairs, streaming) ----------------
            for ct in range(NCT):
                qct = qpool.tile([128, SEQ], f32r, tag="qct")
                nc.sync.dma_start(qct[:], qT_d[ct * 128:(ct + 1) * 128, :])
                hA, hB = 2 * ct, 2 * ct + 1
                for tb in range(NTB):
                    tsl = slice(tb * TBLK, (tb + 1) * TBLK)
                    psoA = psum_o.tile([VW, TBLK], f32, tag="pso")
                    psoB = psum_o.tile([VW, TBLK], f32, tag="pso")
                    for st in range(NST):
                        ssl = slice(st * 128, (st + 1) * 128)
                        psA = psum_a.tile([128, TBLK], f32, tag="ps")
                        nc.tensor.matmul(
                            psA[:], KT[0:64, ct, ssl], qct[0:64, tsl],
                            start=True, stop=True, tile_position=(0, 0),
                        )
                        psB = psum_a.tile([128, TBLK], f32, tag="ps")
                        nc.tensor.matmul(
                            psB[:], KT[64:128, ct, ssl], qct[64:128, tsl],
                            start=True, stop=True, tile_position=(64, 0),
                        )
                        pexA = pexpool.tile([128, TBLK], f32r, tag="pex")
                        nc.scalar.activation(pexA[:], psA[:], AF.Exp)
                        nc.sync.dma_start(attn_d[hA, ssl, tsl], pexA[:].bitcast(f32))
                        nc.tensor.matmul(
                            psoA[:], V[:, st, hA * VW:(hA + 1) * VW], pexA[:],
                            start=(st == 0), stop=(st == NST - 1),
                        )
                        pexB = pexpool.tile([128, TBLK], f32r, tag="pex")
                        nc.scalar.activation(pexB[:], psB[:], AF.Exp)
                        nc.gpsimd.dma_start(attn_d[hB, ssl, tsl], pexB[:].bitcast(f32))
                        nc.tensor.matmul(
                            psoB[:], V[:, st, hB * VW:(hB + 1) * VW], pexB[:],
                            start=(st == 0), stop=(st == NST - 1),
                        )
                    for hp, pso in ((0, psoA), (1, psoB)):
                        hl = 2 * ct + hp
                        prange = slice(hp * 64, hp * 64 + 64)
                        r_row = work.tile([1, TBLK], f32, tag="rrow")
                        nc.vector.reciprocal(r_row[:], pso[HD:VW, :])
                        nc.scalar.dma_start(r_d[hl, tsl], r_row[:])
                        R = work.tile([64, TBLK], f32, tag="R")
                        nc.gpsimd.partition_broadcast(R[:], r_row[:])
                        nc.vector.tensor_tensor(
                            O_sb[prange, ct, tsl], pso[0:HD, :], R[:], Alu.mult
                        )

            # ---------------- output projection ----------------
            w2T_r = wpool.tile([128, NCT, EMBD], f32r, tag="wr")
            nc.sync.dma_start(w2T_r[:], w2T_d[:].rearrange("(ct p) j -> p ct j", p=128))

            for tt in range(NST):
                osb = work.tile([128, EMBD], f32, tag="osb")
                for jb in range(2):
                    ps = psum_a.tile([128, 512], f32, tag="ps")
                    for ct in range(NCT):
                        nc.tensor.matmul(
                            ps[:],
                            O_sb[:, ct, tt * 128:(tt + 1) * 128],
                            w2T_r[:, ct, jb * 512:(jb + 1) * 512],
                            start=(ct == 0), stop=(ct == NCT - 1),
                        )
                    nc.vector.tensor_copy(osb[:, jb * 512:(jb + 1) * 512], ps[:])
                nc.sync.dma_start(outp_d[tt * 128:(tt + 1) * 128, :], osb[:])

    nc.compile()
    return nc


def _get_bass():
    global _BASS
    if _BASS is None:
        _BASS = _build_bass()
    return _BASS


_last_in_maps = None


def kernel(query, key, value, key_w, key_b, value_w, value_b, out_w, out_b,
           nhead=16, **_unused):
    from concourse.bass_utils import run_bass_kernel_spmd

    q = np.ascontiguousarray(np.asarray(query, dtype=np.float32))
    k = np.ascontiguousarray(np.asarray(key, dtype=np.float32))
    v = np.ascontiguousarray(np.asarray(value, dtype=np.float32))
    kw = np.asarray(key_w, dtype=np.float32)
    kb = np.asarray(key_b, dtype=np.float32)
    vw = np.asarray(value_w, dtype=np.float32)
    vb = np.asarray(value_b, dtype=np.float32)
    ow = np.asarray(out_w, dtype=np.float32)
    ob = np.asarray(out_b, dtype=np.float32)

    Q3 = q.reshape(SEQ, 4, EMBD)
    K3 = k.reshape(SEQ, 4, EMBD)
    V3 = v.reshape(SEQ, 4, EMBD)

    in_maps = []
    for core in range(8):
        rb, half = core // 2, core % 2
        cols = slice(half * CLOC, half * CLOC + CLOC)
        in_maps.append({
            "qT": _round_f32r(Q3[:, rb, cols].T),
            "keyT": _round_f32r(K3[:, rb, :].T),
            "valueT": _round_f32r(V3[:, rb, :].T),
            "kwT": _round_f32r(kw[cols, :].T),
            "vwT": _round_f32r(vw[cols, :].T),
            "w2T": _round_f32r(ow[:, cols].T),
            "kb": np.ascontiguousarray(kb[cols].reshape(NCT, 128).T),
            "vb": vb[cols].reshape(1, CLOC).copy(),
        })

    global _last_in_maps
    _last_in_maps = in_maps
    nc = _get_bass()
    res = run_bass_kernel_spmd(nc, in_maps, list(range(8)))

    out2d = np.empty((BSZ * SEQ, EMBD), np.float32)
    attn = np.empty((BSZ * NHEAD, SEQ, SEQ), np.float32)
    for rb in range(4):
        p0 = res.results[2 * rb]["outp"]
        p1 = res.results[2 * rb + 1]["outp"]
        out2d[rb::4] = p0 + p1 + ob[None, :]
        for half in range(2):
            at = res.results[2 * rb + half]["attn_t"]
            rr = res.results[2 * rb + half]["rrow"]
            for hl in range(NHL):
                bh = rb * 16 + half * 8 + hl
                # transpose (s,t)->(t,s) and normalize in one pass
                np.multiply(at[hl].T, rr[hl][:, None], out=attn[bh])
    out = out2d.reshape(BSZ, SEQ, EMBD)
    return out, attn
